# revision 39
# baseline (speedup 1.0000x reference)
"""BERT self-attention (BS=4, SEQ=2048, HID=768, NH=12) on 8 NeuronCores.

Sharding: core c -> batch b = c//2, head-group g = c%2 (6 heads each, as
3 head-pairs j=0..2).

Per-core pipeline (all matmuls fp16, PE fully packed):
  V    = X @ Wv^T + bv              [k,d] layout, 16 k-blocks
  K^T j = Wk_j @ X^T (+bias via ACT Identity copy)   [128d, 2048q]
  Q^T j = Wq_j @ X^T (+bias)                          [128d, 2048q]
  attention per (j, qh in 0..3), q-chunk = 512:
    per kb (16 k-blocks of 128):
      scores: 2 row-tiled MMs (c=64 heads packed at row 0/64) ->
              sab [128k, 1024] f32 PSUM (head A cols 0:512, B 512:1024)
      P = exp(S/8 + mask_bias): even kb -> ACT exp (fp16 out),
          odd kb  -> DVE Schraudolph (tensor_scalar -> int16, bitcast fp16)
      ctx: 2 col-tiled MMs accumulate cab[0:64]/[64:128] over kb
      denom: per kb-pair, 4 col-packed m=1 MMs (ones lhsT) into dn rows
             0/32 (A/B even kb) and 64/96 (A/B odd kb)
    DMA cab [128,512] f32 and 4 dn rows -> DRAM
Host: shard/transposes, final out = ctx / (dn_even + dn_odd), concat.
"""

import numpy as np

import concourse.bass as bass
import concourse.tile as tile
from concourse import bacc
from concourse import mybir
from concourse.bass_utils import run_bass_kernel_spmd

F32 = mybir.dt.float32
F16 = mybir.dt.float16
I16 = mybir.dt.int16
DT_NP = np.float16

BS, SEQ, HID, NH, HD = 4, 2048, 768, 12, 64
NCORES = 8
HPC = 6          # heads per core
FCH = 6          # 128-row chunks of the 768 contraction dim
DSH = HPC * HD   # 384 output features per core
NQH = 4          # q-chunks of 512
QCH = 512

LOG2E = float(np.log2(np.e))
SCH_S1 = 1024.0 * 0.125 * LOG2E          # Schraudolph multiplier
SCH_C = -59.0                             # Schraudolph bias correction
SCH_S2 = 15.0 * 1024.0 + SCH_C           # valid-row add constant
SCH_MASKED = -1.0e6                       # masked-row add (saturates -> -0.0)
ACT_MASKED = -30.0                        # masked exp bias (exp -> 0 in fp16)


def _body(tc, xt_d, wq_d, wk_d, wv_d, bcol_d, mcol_d, scol_d, ot_d, dn_d):
    nc = tc.nc
    Exp = mybir.ActivationFunctionType.Exp
    Ident = mybir.ActivationFunctionType.Identity

    with tc.tile_pool(name="persist", bufs=1) as persist, \
         tc.tile_pool(name="sabp", bufs=3, space="PSUM") as sabp, \
         tc.tile_pool(name="cabp", bufs=1, space="PSUM") as cabp, \
         tc.tile_pool(name="dnp", bufs=1, space="PSUM") as dnp, \
         tc.tile_pool(name="pp", bufs=8) as pp, \
         tc.tile_pool(name="stg", bufs=3) as stg:
        # Warm the exp table set ASAP (overlaps the input DMAs).
        dummy = persist.tile([1, 1], F32, tag="dummy")
        nc.vector.memset(dummy, 0.0)
        nc.scalar.activation(out=dummy, in_=dummy, func=Exp)

        ones = persist.tile([128, 1], F16, tag="ones")
        nc.vector.memset(ones, 1.0)
        wrm = persist.tile([128, 512], F16, tag="wrm")
        nc.vector.memset(wrm, 0.0)

        # Input DMAs, balanced over the 3 DMA-capable queues (~26 GB/s per
        # queue): wv first on gpsimd (V runs first), x round-robin over all
        # three, then wk/wq split, small tiles last.
        # x + wv ride the gpsimd queue: SWDGE fans a transfer out across all
        # 16 SDMA engines, far faster than the per-engine HWDGE rings that
        # serve the sync/scalar queues (which carry wk/wq).
        xts, wvl = [], []
        for f in range(FCH):
            w = persist.tile([128, DSH], F16, tag=f"wv{f}", name=f"wv{f}")
            nc.gpsimd.dma_start(out=w, in_=wv_d[f * 128:(f + 1) * 128, :])
            wvl.append(w)
            t = persist.tile([128, SEQ], F16, tag=f"x{f}", name=f"x{f}")
            nc.gpsimd.dma_start(out=t, in_=xt_d[f * 128:(f + 1) * 128, :])
            xts.append(t)
        wkl, wql = [], []
        for lst, dram, nm, eng in ((wkl, wk_d, "wk", nc.sync),
                                   (wql, wq_d, "wq", nc.scalar)):
            for f in range(FCH):
                w = persist.tile([128, DSH], F16, tag=f"{nm}{f}",
                                 name=f"{nm}{f}")
                eng.dma_start(out=w, in_=dram[f * 128:(f + 1) * 128, :])
                lst.append(w)
        bcol = persist.tile([128, 6], F32, tag="bcol")
        mcol = persist.tile([128, 16], F32, tag="mcol")
        scol = persist.tile([128, 16], F32, tag="scol")
        nc.scalar.dma_start(out=bcol, in_=bcol_d[:, :])
        nc.scalar.dma_start(out=mcol, in_=mcol_d[:, :])
        nc.gpsimd.dma_start(out=scol, in_=scol_d[:, :])

        qt = [persist.tile([128, SEQ], F16, tag=f"qt{j}", name=f"qt{j}") for j in range(3)]
        kt = [persist.tile([128, SEQ], F16, tag=f"kt{j}", name=f"kt{j}") for j in range(3)]
        vt = persist.tile([128, 16, DSH], F16, tag="vt")

        # HAM warm-up: keep the PE busy during the input-DMA ramp so the
        # V projection runs at 2.4 GHz instead of the cold 1.2 GHz clock.
        wps = sabp.tile([128, 1024], F32, tag="sab", name="wps")
        for _ in range(28):
            nc.tensor.matmul(wps[:, 0:512], lhsT=wrm[:, 0:128], rhs=wrm,
                             start=True, stop=True)

        # ---------------- V projection, pass A: f0..f4 --------------------
        # The f5 term is added in pass B (after K0/Q0) so the PE queue never
        # head-of-line blocks on the last x chunk; the DVE adds hide under
        # the remaining projection blocks.
        for kb in range(16):
            ks = slice(kb * 128, (kb + 1) * 128)
            ps = sabp.tile([128, 1024], F32, tag="sab", name="vps")
            for f in range(FCH - 1):
                nc.tensor.matmul(ps[:, 0:DSH], lhsT=xts[f][:, ks], rhs=wvl[f],
                                 start=(f == 0), stop=(f == FCH - 2))
            nc.vector.tensor_copy(out=vt[:, kb, :], in_=ps[:, 0:DSH])

        # ---------------- per head-pair: K/Q projection then attention ----
        # Producer (scores+exp) runs 2 kb ahead of consumer (ctx/dn); the
        # pipeline is continuous across qh AND j boundaries so the PE never
        # drains. Consumers for kb=14/15 are popped together so the stage
        # copies (and the dn/cab buffer frees) land one kb earlier.
        pending = []  # list of (kb, emit_fn)

        def pop_one():
            pending.pop(0)[1]()

        def emit_proj(j):
            js = slice(j * 128, (j + 1) * 128)
            # f-outer / qc-inner: 4 consecutive matmuls share the same lhsT
            # (weight reuse), two [128,1024] PSUM tiles = 4 accumulators.
            for wl, dst, bc in ((wkl, kt[j], 3 + j), (wql, qt[j], j)):
                pst = [sabp.tile([128, 1024], F32, tag="sab", name="qkps")
                       for _ in range(2)]
                for f in range(FCH):
                    for qc in range(4):
                        nc.tensor.matmul(
                            pst[qc // 2][:, (qc % 2) * 512:(qc % 2) * 512 + 512],
                            lhsT=wl[f][:, js],
                            rhs=xts[f][:, qc * 512:(qc + 1) * 512],
                            start=(f == 0), stop=(f == FCH - 1),
                            skip_group_check=True)
                for qc in range(4):
                    src = pst[qc // 2][:, (qc % 2) * 512:(qc % 2) * 512 + 512]
                    if qc % 2 == 0:
                        nc.scalar.activation(
                            out=dst[:, qc * 512:(qc + 1) * 512], in_=src,
                            func=Ident, bias=bcol[:, bc:bc + 1], scale=1.0)
                    else:
                        nc.vector.tensor_scalar_add(
                            out=dst[:, qc * 512:(qc + 1) * 512], in0=src,
                            scalar1=bcol[:, bc:bc + 1])

        def make_ctx(cab, dnt, p_cur, p_prev, kb, j, qh):
            hA, hB = 2 * j, 2 * j + 1

            def emit():
                st, sp_ = (kb == 0), (kb == 15)
                nc.tensor.matmul(cab[0:64, :],
                                 lhsT=vt[:, kb, hA * 64:hA * 64 + 64],
                                 rhs=p_cur[:, 0:512], start=st, stop=sp_,
                                 skip_group_check=True)
                nc.tensor.matmul(cab[64:128, :],
                                 lhsT=vt[:, kb, hB * 64:hB * 64 + 64],
                                 rhs=p_cur[:, 512:1024], start=st,
                                 stop=sp_, skip_group_check=True)
                if kb % 2 == 1:
                    # all four rhs ready together -> one 4x col group
                    st2, sp2 = (kb == 1), (kb == 15)
                    for r, rhs in ((0, p_cur[:, 0:512]),
                                   (32, p_cur[:, 512:1024]),
                                   (64, p_prev[:, 0:512]),
                                   (96, p_prev[:, 512:1024])):
                        nc.tensor.matmul(dnt[r:r + 1, :], lhsT=ones,
                                         rhs=rhs, start=st2, stop=sp2,
                                         tile_position=(0, r),
                                         skip_group_check=True)
                if kb == 15:
                    cstage = stg.tile([128, QCH], F16, tag="cst",
                                      name="cstage")
                    nc.scalar.copy(out=cstage, in_=cab)
                    nc.gpsimd.dma_start(out=ot_d[j, qh], in_=cstage)
                    dstage = stg.tile([97, QCH], F16, tag="dstage",
                                      name="dstage")
                    nc.vector.tensor_copy(out=dstage, in_=dnt[0:97, :])
                    nc.scalar.dma_start(out=dn_d[j, qh], in_=dstage)
            return emit

        emit_proj(0)
        # ---------------- V projection, pass B: += x5 * wv5 ----------------
        for kb in range(16):
            ks = slice(kb * 128, (kb + 1) * 128)
            ps = sabp.tile([128, 1024], F32, tag="sab", name="vps2")
            nc.tensor.matmul(ps[:, 0:DSH], lhsT=xts[FCH - 1][:, ks],
                             rhs=wvl[FCH - 1], start=True, stop=True)
            nc.vector.tensor_add(out=vt[:, kb, :], in0=vt[:, kb, :],
                                 in1=ps[:, 0:DSH])
        emit_proj(1)
        emit_proj(2)
        for j in range(3):
            for qh in range(NQH):
                qs = slice(qh * QCH, (qh + 1) * QCH)
                cab = cabp.tile([128, QCH], F32, tag="cab", name="cab")
                dnt = dnp.tile([128, QCH], F32, tag="dn", name="dn")
                p_prev = None
                for kb in range(16):
                    ks = slice(kb * 128, (kb + 1) * 128)
                    sab = sabp.tile([128, 1024], F32, tag="sab", name="sab")
                    nc.tensor.matmul(sab[:, 0:512], lhsT=kt[j][0:64, ks],
                                     rhs=qt[j][0:64, qs],
                                     start=True, stop=True)
                    nc.tensor.matmul(sab[:, 512:1024], lhsT=kt[j][64:128, ks],
                                     rhs=qt[j][64:128, qs],
                                     start=True, stop=True)
                    p = pp.tile([128, 1024], F16, tag="p", name="ptile")
                    if kb % 2 == 0:
                        nc.scalar.activation(out=p, in_=sab, func=Exp,
                                             scale=0.125,
                                             bias=mcol[:, kb:kb + 1])
                    else:
                        nc.vector.tensor_scalar(
                            out=p.bitcast(I16), in0=sab,
                            scalar1=SCH_S1, scalar2=scol[:, kb:kb + 1],
                            op0=mybir.AluOpType.mult, op1=mybir.AluOpType.add)
                    pending.append((kb, make_ctx(cab, dnt, p, p_prev,
                                                 kb, j, qh)))
                    p_prev = p
                    # pop consumers in kb-pairs so the PE sees
                    # [scores,scores][ctx,ctx,dn] super-steps: the
                    # scores->scores transition is row-disjoint and cheap
                    if kb % 2 == 1:
                        while len(pending) > 2:
                            pop_one()
        while pending:
            pop_one()


def build_nc():
    nc = bacc.Bacc("TRN2")
    xt_d = nc.declare_dram_parameter("xt", [HID, SEQ], F16, isOutput=False)
    wq_d = nc.declare_dram_parameter("wqT", [HID, DSH], F16, isOutput=False)
    wk_d = nc.declare_dram_parameter("wkT", [HID, DSH], F16, isOutput=False)
    wv_d = nc.declare_dram_parameter("wvT", [HID, DSH], F16, isOutput=False)
    bcol_d = nc.declare_dram_parameter("bcol", [128, 6], F32, isOutput=False)
    mcol_d = nc.declare_dram_parameter("mcol", [128, 16], F32, isOutput=False)
    scol_d = nc.declare_dram_parameter("scol", [128, 16], F32, isOutput=False)
    ot_d = nc.declare_dram_parameter("OT", [3, NQH, 128, QCH], F16,
                                     isOutput=True)
    dn_d = nc.declare_dram_parameter("DN", [3, NQH, 97, QCH], F16,
                                     isOutput=True)
    with tile.TileContext(nc) as tc:
        _body(tc, xt_d, wq_d, wk_d, wv_d, bcol_d, mcol_d, scol_d, ot_d, dn_d)
    nc.finalize()
    return nc


_NC_CACHE = None


def _get_nc():
    global _NC_CACHE
    if _NC_CACHE is None:
        _NC_CACHE = build_nc()
    return _NC_CACHE


def make_in_maps(hidden_states, attention_mask, Wq, bq, Wk, bk, Wv, bv):
    in_maps = []
    for c in range(NCORES):
        b, g = c // 2, c % 2
        hs = slice(g * DSH, (g + 1) * DSH)
        xt = np.ascontiguousarray(hidden_states[b].T.astype(DT_NP))
        keep = np.asarray(attention_mask[b, 0, 0]) > -1       # [SEQ]
        keep_kb = keep.reshape(16, 128).T                     # [128, 16]
        mcol = np.where(keep_kb, 0.0, ACT_MASKED).astype(np.float32)
        scol = np.where(keep_kb, SCH_S2, SCH_MASKED).astype(np.float32)
        # Q bias cols 0..2, K bias cols 3..5 (per 128-d j-tile)
        bcol = np.empty((128, 6), np.float32)
        for j in range(3):
            bcol[:, j] = bq[hs][j * 128:(j + 1) * 128]
            bcol[:, 3 + j] = bk[hs][j * 128:(j + 1) * 128]

        def aug(W):
            return np.ascontiguousarray(W[hs, :].T.astype(DT_NP))

        in_maps.append({
            "xt": xt,
            "wqT": aug(Wq),
            "wkT": aug(Wk),
            "wvT": aug(Wv),
            "bcol": bcol,
            "mcol": np.ascontiguousarray(mcol),
            "scol": np.ascontiguousarray(scol),
        })
    return in_maps


def gather_out(results, bv):
    out = np.empty((BS, SEQ, HID), np.float32)
    for c in range(NCORES):
        b, g = c // 2, c % 2
        ot = results[c]["OT"].astype(np.float32)   # [3, 4, 128, 512]
        dn = results[c]["DN"].astype(np.float32)   # [3, 4, 97, 512]
        for j in range(3):
            den_a = (dn[j, :, 0, :] + dn[j, :, 64, :]).reshape(SEQ)  # [2048]
            den_b = (dn[j, :, 32, :] + dn[j, :, 96, :]).reshape(SEQ)
            # ctx rows 0:64 = head 2j, 64:128 = head 2j+1; [4,64,512]->[2048,64]
            ctx_a = ot[j, :, 0:64, :].transpose(0, 2, 1).reshape(SEQ, HD)
            ctx_b = ot[j, :, 64:128, :].transpose(0, 2, 1).reshape(SEQ, HD)
            c0 = g * DSH + (2 * j) * HD
            out[b, :, c0:c0 + HD] = ctx_a / den_a[:, None] + bv[c0:c0 + HD]
            out[b, :, c0 + HD:c0 + 2 * HD] = (ctx_b / den_b[:, None]
                                              + bv[c0 + HD:c0 + 2 * HD])
    return out


def kernel(hidden_states, attention_mask, Wq, bq, Wk, bk, Wv, bv):
    nc = _get_nc()
    in_maps = make_in_maps(hidden_states, attention_mask,
                           Wq, bq, Wk, bk, Wv, bv)
    res = run_bass_kernel_spmd(nc, in_maps, core_ids=list(range(NCORES)))
    return gather_out(res.results, np.asarray(bv, np.float32))
